# revision 1
# baseline (speedup 1.0000x reference)
"""Trainium2 Bass kernel for DepthWiseSeparableConv (shared-3x3 dw conv + BN+ReLU + 1x1 conv + BN+ReLU).

Strategy (8 NeuronCores, data-parallel over batch N=32 -> 4 images/core):
  - x and z travel as fp16 on the wire (host casts); all matmuls fp16
    with fp32 PSUM accumulation -> halves HBM traffic at ~1e-3 accuracy.
  - PE transpose (regular matmul) to x^T chunks [112 pix, c]; the moving
    operand is block-diag(s1) so BN1's scale is applied for free here.
  - Depthwise 3x3 conv as banded matmuls, scatter form: for each source
    chunk (stationary = x^T chunk), matmuls against [B+1|B0|B-1] slices
    accumulate into PSUM pair tiles -> output lands directly in [c, pix].
  - BN1 shift + ReLU: one op (bias-add + max0), alternating ScalarE/VectorE,
    cast to fp16 -> y [c, pix].
  - 1x1 conv = GEMM over 2 c-tiles; BN2's scale is folded into the weights
    (inside the relu argument, so no sign assumption), shift+ReLU as 1 op.
  - z fp16 -> two half-otile DMAs out; host upcasts to fp32.

Self-contained: hardcodes all shapes; no file reads.
"""

import numpy as np

N, C, CO, H, W = 32, 256, 512, 56, 56
EPS = 1e-5
N_CORES = 8
NPC = N // N_CORES      # images per core
HW = H * W              # 3136
CH = 112                # pixel chunk = 2 rows of 56
NCHUNK = HW // CH       # 28
NPAIR = NCHUNK // 2     # 14
CT = C // 128           # 2 c-tiles
OT = CO // 128          # 4 o-tiles
GN = 448                # gemm pixel-block
NPB = HW // GN          # 7

_cache = {}


def _build_program():
    import concourse.mybir as mybir
    import concourse.tile as tile
    from concourse import bacc

    f32 = mybir.dt.float32
    f16 = mybir.dt.float16

    nc = bacc.Bacc("TRN2", target_bir_lowering=False, debug=False)

    x_d = nc.dram_tensor("x", [NPC, C, H, W], f16, kind="ExternalInput").ap()
    bmat_d = nc.dram_tensor("bmat", [CH, 3 * CH], f16, kind="ExternalInput").ap()
    # block-diagonal scale: sdiag[:, ci*128:(ci+1)*128] = diag(s1[ci-tile])
    sdiag_d = nc.dram_tensor("sdiag", [128, CT * 128], f16, kind="ExternalInput").ap()
    wT_d = nc.dram_tensor("wT", [C, CO], f16, kind="ExternalInput").ap()  # s2-folded
    t1_d = nc.dram_tensor("t1", [128, CT], f32, kind="ExternalInput").ap()
    t2_d = nc.dram_tensor("t2", [128, OT], f32, kind="ExternalInput").ap()
    z_d = nc.dram_tensor("z", [NPC, CO, H, W], f16, kind="ExternalOutput").ap()

    relu = mybir.ActivationFunctionType.Relu
    add = mybir.AluOpType.add
    amax = mybir.AluOpType.max

    with tile.TileContext(nc) as tc:
        with (
            tc.tile_pool(name="singles", bufs=1) as singles,
            tc.tile_pool(name="xp", bufs=3) as xp,
            tc.tile_pool(name="xtp", bufs=2) as xtp,
            tc.tile_pool(name="yp", bufs=3) as yp,
            tc.tile_pool(name="zp", bufs=6) as zp,
            tc.tile_pool(name="tps", bufs=2, space="PSUM") as tps,
            tc.tile_pool(name="cvs", bufs=3, space="PSUM") as cvs,
            tc.tile_pool(name="zps", bufs=3, space="PSUM") as zps,
        ):
            # inputs ride the ScalarE HWDGE ring; outputs + singles the SP ring,
            # so per-ring FIFO order can't stall loads behind stores
            sdiag_sb = singles.tile([128, CT * 128], f16)
            nc.scalar.dma_start(out=sdiag_sb, in_=sdiag_d)
            bmat_sb = singles.tile([CH, 3 * CH], f16)
            nc.sync.dma_start(out=bmat_sb, in_=bmat_d)
            w_sb = singles.tile([128, CT, CO], f16)
            for ci in range(CT):
                nc.sync.dma_start(out=w_sb[:, ci, :], in_=wT_d[128 * ci:128 * (ci + 1), :])
            t1_sb = singles.tile([128, CT], f32)
            nc.sync.dma_start(out=t1_sb, in_=t1_d)
            t2_sb = singles.tile([128, OT], f32)
            nc.sync.dma_start(out=t2_sb, in_=t2_d)

            # epilogue helper: out = relu(in + bias[p]) on alternating engines
            epi_ctr = [0]

            def epilogue(out_ap, in_ap, bias_ap):
                use_act = (epi_ctr[0] % 2 == 1)
                epi_ctr[0] += 1
                if bias_ap is None:
                    if use_act:
                        nc.scalar.copy(out=out_ap, in_=in_ap)
                    else:
                        nc.vector.tensor_copy(out_ap, in_ap)
                elif use_act:
                    nc.scalar.activation(out=out_ap, in_=in_ap, func=relu,
                                         bias=bias_ap, scale=1.0)
                else:
                    nc.vector.tensor_scalar(out=out_ap, in0=in_ap,
                                            scalar1=bias_ap, scalar2=0.0,
                                            op0=add, op1=amax)

            # conv moving-operand slices of bmat = [B+1 | B0 | B-1]
            A_even = bmat_sb[:, CH:3 * CH]       # [B0 | B-1]
            A_odd = bmat_sb[:, 0:2 * CH]         # [B+1 | B0]
            B_plus = bmat_sb[:, 0:CH]            # B+1 (from even source s -> chunk s-1)
            B_minus = bmat_sb[:, 2 * CH:3 * CH]  # B-1 (from odd source s -> chunk s+1)

            for img in range(NPC):
                # ---- stage A: load x, cast fp32->f16 during DMA ----
                # 16 pad cols so transpose stationaries can read 128 cols (FWL)
                x_sb = xp.tile([128, CT, HW + 16], f16, tag="x")
                for ci in range(CT):
                    nc.vector.memset(x_sb[:, ci, HW:], 0.0)
                xflats = [x_d[img, 128 * ci:128 * (ci + 1), :, :].rearrange("c h w -> c (h w)")
                          for ci in range(CT)]
                QTR = HW // 4
                for hh in range(4):      # quarters, both c-tiles interleaved,
                    for ci in range(CT):  # so the first transposes start early
                        nc.scalar.dma_start(
                            out=x_sb[:, ci, QTR * hh: QTR * (hh + 1)],
                            in_=xflats[ci][:, QTR * hh: QTR * (hh + 1)],
                        )

                # ---- stage B: transpose (x^T scaled by s1 via block-diag rhs) ----
                xt_sb = xtp.tile([CH, NCHUNK * C], f16, tag="xt")
                for q in range(NPAIR):
                    # stationary reads 128 cols (112 real + 16 overlap/pad) so
                    # FWL engages; psum rows 112:128 are written but never read
                    t_ps = tps.tile([128, 512], f32, tag="tps")
                    for k in range(2):          # chunk s = 2q+k
                        s = 2 * q + k
                        for ci in range(CT):
                            nc.tensor.matmul(
                                t_ps[:, 256 * k + 128 * ci: 256 * k + 128 * (ci + 1)],
                                lhsT=x_sb[:, ci, CH * s: CH * s + 128],
                                rhs=sdiag_sb[:, 128 * ci:128 * (ci + 1)],
                                start=True, stop=True,
                            )
                    epilogue(xt_sb[:, 512 * q: 512 * (q + 1)], t_ps[0:CH, :], None)

                def xt(s, ci):
                    return xt_sb[:, 256 * s + 128 * ci: 256 * s + 128 * (ci + 1)]

                # ---- stage C: depthwise conv, scatter form over sources ----
                # combined psum tile g covers chunks 4g..4g+3 as [128, bank 2, 256]
                y_sb = yp.tile([128, CT, HW], f16, tag="y")
                NG = NCHUNK // 4  # 7
                B_full = bmat_sb[:, 0:3 * CH]  # [B+1 | B0 | B-1], 336 wide
                for ci in range(CT):
                    for g in range(NG):
                        # one 2 KiB PSUM bank holds 4 chunks (448 of 512 f32);
                        # interior sources then land their full 336-wide
                        # [B+1|B0|B-1] window in a single matmul. Every MM
                        # overlaps its predecessor's range, which both orders
                        # them and makes the per-element accumulate correct.
                        cv = cvs.tile([128, 512], f32, tag="cv")
                        mms = []  # (src_chunk, rhs, lo, hi)
                        if g > 0:
                            mms.append((4 * g - 1, B_minus, 0, CH))
                        mms.append((4 * g, A_even, 0, 2 * CH))
                        mms.append((4 * g + 1, B_full, 0, 3 * CH))
                        mms.append((4 * g + 2, B_full, CH, 4 * CH))
                        mms.append((4 * g + 3, A_odd, 2 * CH, 4 * CH))
                        if g < NG - 1:
                            mms.append((4 * g + 4, B_plus, 3 * CH, 4 * CH))
                        for i, (s, rhs, lo, hi) in enumerate(mms):
                            nc.tensor.matmul(
                                cv[:, lo:hi], lhsT=xt(s, ci), rhs=rhs,
                                start=(i == 0), stop=(i == len(mms) - 1),
                                skip_group_check=True,
                            )
                        epilogue(
                            y_sb[:, ci, 448 * g: 448 * (g + 1)],
                            cv[:, 0:4 * CH],
                            t1_sb[:, ci:ci + 1],
                        )

                # ---- stage D: pointwise GEMM + BN2+ReLU (s2 folded into W) ----
                for oi in range(OT):
                    z_sb = zp.tile([128, HW], f16, tag="z")
                    zflat = z_d[img, 128 * oi:128 * (oi + 1), :, :].rearrange("o h w -> o (h w)")
                    for pb in range(NPB):
                        z_ps = zps.tile([128, GN], f32, tag="zps")
                        for ci in range(CT):
                            nc.tensor.matmul(
                                z_ps,
                                lhsT=w_sb[:, ci, 128 * oi:128 * (oi + 1)],
                                rhs=y_sb[:, ci, GN * pb: GN * (pb + 1)],
                                start=(ci == 0), stop=(ci == CT - 1),
                            )
                        epilogue(z_sb[:, GN * pb: GN * (pb + 1)], z_ps,
                                 t2_sb[:, oi:oi + 1])
                        # ---- stage E: two half-DMAs per otile (few descriptors,
                        # still overlaps the second half's compute) ----
                        if pb == 3:
                            nc.sync.dma_start(out=zflat[:, 0:4 * GN], in_=z_sb[:, 0:4 * GN])
                        elif pb == NPB - 1:
                            nc.sync.dma_start(out=zflat[:, 4 * GN:], in_=z_sb[:, 4 * GN:])

    nc.compile()
    return nc


def _build_bmats(k2d):
    """B[t][p,j] = k2d[1+dh, 1+dw] with dh = 2t + p//56 - j//56, dw = p%56 - j%56."""
    p = np.arange(CH)
    j = np.arange(CH)
    ph, pw = p // W, p % W
    jh, jw = j // W, j % W
    out = []
    for t in (1, 0, -1):  # concat order [B_{+1} | B_0 | B_{-1}]
        dh = 2 * t + ph[:, None] - jh[None, :]
        dw = pw[:, None] - jw[None, :]
        ok = (np.abs(dh) <= 1) & (np.abs(dw) <= 1)
        B = np.where(ok, k2d[np.clip(1 + dh, 0, 2), np.clip(1 + dw, 0, 2)], 0.0)
        out.append(B)
    return np.concatenate(out, axis=1).astype(np.float32)  # [112, 336]


def _get_runner(nc):
    """Build a cached jitted shard_map executable mirroring
    concourse.bass2jax.run_bass_via_pjrt (which re-traces on every call)."""
    import jax
    import jax.numpy as jnp
    import concourse.mybir as mybir
    from jax.sharding import Mesh, PartitionSpec
    from jax.experimental.shard_map import shard_map
    from concourse.bass2jax import (
        _bass_exec_p, install_neuronx_cc_hook, partition_id_tensor)

    install_neuronx_cc_hook()

    partition_name = nc.partition_id_tensor.name if nc.partition_id_tensor else None

    in_names, out_names, out_avals = [], [], []
    for alloc in nc.m.functions[0].allocations:
        if not isinstance(alloc, mybir.MemoryLocationSet):
            continue
        name = alloc.memorylocations[0].name
        if alloc.kind == "ExternalInput":
            if name != partition_name:
                in_names.append(name)
        elif alloc.kind == "ExternalOutput":
            out_names.append(name)
            out_avals.append(jax.core.ShapedArray(
                tuple(alloc.tensor_shape), mybir.dt.np(alloc.dtype)))
    n_params = len(in_names)
    all_names = list(in_names) + list(out_names)
    if partition_name is not None:
        all_names.append(partition_name)

    def _body(*args):
        operands = list(args)
        if partition_name is not None:
            operands.append(partition_id_tensor())
        return tuple(_bass_exec_p.bind(
            *operands,
            out_avals=tuple(out_avals),
            in_names=tuple(all_names),
            out_names=tuple(out_names),
            lowering_input_output_aliases=(),
            sim_require_finite=True,
            sim_require_nnan=True,
            nc=nc,
        ))

    n_outs = len(out_avals)
    devices = jax.devices()[:N_CORES]
    mesh = Mesh(np.asarray(devices), ("core",))
    fn = jax.jit(
        shard_map(
            _body, mesh=mesh,
            in_specs=(PartitionSpec("core"),) * (n_params + n_outs),
            out_specs=(PartitionSpec("core"),) * len(out_names),
            check_rep=False,
        ),
        donate_argnums=tuple(range(n_params, n_params + n_outs)),
        keep_unused=True,
    )
    out_shapes = [(N_CORES * a.shape[0], *a.shape[1:]) for a in out_avals]
    out_dtypes = [a.dtype for a in out_avals]
    return fn, in_names, out_names, out_shapes, out_dtypes


def _prep_inputs(x, w_dw, b_dw, bn1_gamma, bn1_beta, bn1_mean, bn1_var,
                 w_pw, b_pw, bn2_gamma, bn2_beta, bn2_mean, bn2_var):
    bf = np.float16  # fp16 on the wire and on-chip: halves HBM traffic vs fp32

    x = np.asarray(x, np.float32)
    s1 = (bn1_gamma / np.sqrt(bn1_var + EPS)).astype(np.float32)
    t1 = (bn1_beta - bn1_mean * s1 + s1 * float(np.asarray(b_dw).reshape(-1)[0])).astype(np.float32)
    s2 = (bn2_gamma / np.sqrt(bn2_var + EPS)).astype(np.float32)
    t2 = (bn2_beta - bn2_mean * s2 + s2 * np.asarray(b_pw, np.float32)).astype(np.float32)

    # s1 applied during the transpose matmul (block-diag rhs);
    # s2 folded into the pointwise weights (inside relu arg, sign-free).
    sdiag = np.zeros((128, CT * 128), np.float32)
    for ci in range(CT):
        sdiag[:, 128 * ci:128 * (ci + 1)] = np.diag(s1[128 * ci:128 * (ci + 1)])
    wS = np.asarray(w_pw, np.float32) * s2[:, None]          # [CO, C]

    shared = {
        "bmat": _build_bmats(np.asarray(w_dw, np.float32)[0, 0]).astype(bf),
        "sdiag": sdiag.astype(bf),
        "wT": np.ascontiguousarray(wS.T).astype(bf),
        "t1": np.ascontiguousarray(t1.reshape(CT, 128).T),
        "t2": np.ascontiguousarray(t2.reshape(OT, 128).T),
    }
    return [{"x": np.ascontiguousarray(x[NPC * i: NPC * (i + 1)]).astype(np.float16),
             **shared} for i in range(N_CORES)]


def kernel(x, w_dw, b_dw, bn1_gamma, bn1_beta, bn1_mean, bn1_var,
           w_pw, b_pw, bn2_gamma, bn2_beta, bn2_mean, bn2_var):
    if "nc" not in _cache:
        _cache["nc"] = _build_program()
    nc = _cache["nc"]

    in_maps = _prep_inputs(x, w_dw, b_dw, bn1_gamma, bn1_beta, bn1_mean, bn1_var,
                           w_pw, b_pw, bn2_gamma, bn2_beta, bn2_mean, bn2_var)
    try:
        # cached-jit PJRT path only under axon (native NRT boxes take the
        # run_bass_kernel_spmd path below, which drives /dev/neuron* directly)
        from concourse._compat import axon_active
        if not axon_active():
            raise RuntimeError("native NRT environment")
        if "runner" not in _cache:
            _cache["runner"] = _get_runner(nc)
        fn, in_names, out_names, out_shapes, out_dtypes = _cache["runner"]
        concat_in = [np.concatenate([m[name] for m in in_maps], axis=0)
                     for name in in_names]
        zeros = [np.zeros(s, d) for s, d in zip(out_shapes, out_dtypes)]
        outs = fn(*concat_in, *zeros)
        z = np.asarray(outs[out_names.index("z")])
        return z.astype(np.float32)
    except Exception:
        # fallback + retry: rare transient NRT_EXEC_UNIT_UNRECOVERABLE errors
        # have been observed on first device contact
        import time
        from concourse import bass_utils
        last_exc = None
        for attempt in range(3):
            try:
                res = bass_utils.run_bass_kernel_spmd(
                    nc, in_maps, core_ids=list(range(N_CORES)))
                z = np.concatenate([res.results[i]["z"] for i in range(N_CORES)], axis=0)
                return np.asarray(z, np.float32)
            except Exception as e:
                last_exc = e
                time.sleep(2.0 * (attempt + 1))
        raise last_exc


def _kernel_spmd_reference_path(x, w_dw, b_dw, bn1_gamma, bn1_beta, bn1_mean, bn1_var,
                                w_pw, b_pw, bn2_gamma, bn2_beta, bn2_mean, bn2_var):
    """Original path through bass_utils.run_bass_kernel_spmd (kept for checking)."""
    from concourse import bass_utils

    if "nc" not in _cache:
        _cache["nc"] = _build_program()
    nc = _cache["nc"]

    in_maps = _prep_inputs(x, w_dw, b_dw, bn1_gamma, bn1_beta, bn1_mean, bn1_var,
                           w_pw, b_pw, bn2_gamma, bn2_beta, bn2_mean, bn2_var)
    res = bass_utils.run_bass_kernel_spmd(nc, in_maps, core_ids=list(range(N_CORES)))
    z = np.concatenate([res.results[i]["z"] for i in range(N_CORES)], axis=0)
    return np.asarray(z, np.float32)



# revision 3
# speedup vs baseline: 143.1756x; 143.1756x over previous
"""Trainium2 Bass kernel for DepthWiseSeparableConv (shared-3x3 dw conv + BN+ReLU + 1x1 conv + BN+ReLU).

Strategy (8 NeuronCores, data-parallel over batch N=32 -> 4 images/core):
  - x and z travel as fp16 on the wire (host casts); all matmuls fp16
    with fp32 PSUM accumulation -> halves HBM traffic at ~1e-3 accuracy.
  - PE transpose (regular matmul) to x^T chunks [112 pix, c]; the moving
    operand is block-diag(s1) so BN1's scale is applied for free here.
  - Depthwise 3x3 conv as banded matmuls, scatter form: for each source
    chunk (stationary = x^T chunk), matmuls against [B+1|B0|B-1] slices
    accumulate into PSUM pair tiles -> output lands directly in [c, pix].
  - BN1 shift + ReLU: one op (bias-add + max0), alternating ScalarE/VectorE,
    cast to fp16 -> y [c, pix].
  - 1x1 conv = GEMM over 2 c-tiles; BN2's scale is folded into the weights
    (inside the relu argument, so no sign assumption), shift+ReLU as 1 op.
  - z fp16 -> two half-otile DMAs out; host upcasts to fp32.

Self-contained: hardcodes all shapes; no file reads.
"""

import numpy as np

N, C, CO, H, W = 32, 256, 512, 56, 56
EPS = 1e-5
N_CORES = 8
NPC = N // N_CORES      # images per core
HW = H * W              # 3136
CH = 112                # pixel chunk = 2 rows of 56
NCHUNK = HW // CH       # 28
NPAIR = NCHUNK // 2     # 14
CT = C // 128           # 2 c-tiles
OT = CO // 128          # 4 o-tiles
GN = 448                # gemm pixel-block
NPB = HW // GN          # 7

_cache = {}


def _build_program():
    import concourse.mybir as mybir
    import concourse.tile as tile
    from concourse import bacc

    f32 = mybir.dt.float32
    f16 = mybir.dt.float16

    nc = bacc.Bacc("TRN2", target_bir_lowering=False, debug=False)

    x_d = nc.dram_tensor("x", [NPC, C, H, W], f16, kind="ExternalInput").ap()
    bmat_d = nc.dram_tensor("bmat", [CH, 3 * CH], f16, kind="ExternalInput").ap()
    # block-diagonal scale: sdiag[:, ci*128:(ci+1)*128] = diag(s1[ci-tile])
    sdiag_d = nc.dram_tensor("sdiag", [128, CT * 128], f16, kind="ExternalInput").ap()
    wT_d = nc.dram_tensor("wT", [C, CO], f16, kind="ExternalInput").ap()  # s2-folded
    t1_d = nc.dram_tensor("t1", [128, CT], f32, kind="ExternalInput").ap()
    t2_d = nc.dram_tensor("t2", [128, OT], f32, kind="ExternalInput").ap()
    z_d = nc.dram_tensor("z", [NPC, CO, H, W], f16, kind="ExternalOutput").ap()

    relu = mybir.ActivationFunctionType.Relu
    add = mybir.AluOpType.add
    amax = mybir.AluOpType.max

    with tile.TileContext(nc) as tc:
        with (
            tc.tile_pool(name="singles", bufs=1) as singles,
            tc.tile_pool(name="xp", bufs=3) as xp,
            tc.tile_pool(name="xtp", bufs=2) as xtp,
            tc.tile_pool(name="yp", bufs=3) as yp,
            tc.tile_pool(name="zp", bufs=6) as zp,
            tc.tile_pool(name="tps", bufs=2, space="PSUM") as tps,
            tc.tile_pool(name="cvs", bufs=3, space="PSUM") as cvs,
            tc.tile_pool(name="zps", bufs=3, space="PSUM") as zps,
        ):
            # inputs ride the ScalarE HWDGE ring; outputs + singles the SP ring,
            # so per-ring FIFO order can't stall loads behind stores
            sdiag_sb = singles.tile([128, CT * 128], f16)
            nc.scalar.dma_start(out=sdiag_sb, in_=sdiag_d)
            bmat_sb = singles.tile([CH, 3 * CH], f16)
            nc.sync.dma_start(out=bmat_sb, in_=bmat_d)
            w_sb = singles.tile([128, CT, CO], f16)
            for ci in range(CT):
                nc.sync.dma_start(out=w_sb[:, ci, :], in_=wT_d[128 * ci:128 * (ci + 1), :])
            t1_sb = singles.tile([128, CT], f32)
            nc.sync.dma_start(out=t1_sb, in_=t1_d)
            t2_sb = singles.tile([128, OT], f32)
            nc.sync.dma_start(out=t2_sb, in_=t2_d)

            # epilogue helper: out = relu(in + bias[p]) on alternating engines
            epi_ctr = [0]

            def epilogue(out_ap, in_ap, bias_ap):
                use_act = (epi_ctr[0] % 2 == 1)
                epi_ctr[0] += 1
                if bias_ap is None:
                    if use_act:
                        nc.scalar.copy(out=out_ap, in_=in_ap)
                    else:
                        nc.vector.tensor_copy(out_ap, in_ap)
                elif use_act:
                    nc.scalar.activation(out=out_ap, in_=in_ap, func=relu,
                                         bias=bias_ap, scale=1.0)
                else:
                    nc.vector.tensor_scalar(out=out_ap, in0=in_ap,
                                            scalar1=bias_ap, scalar2=0.0,
                                            op0=add, op1=amax)

            # conv moving-operand slices of bmat = [B+1 | B0 | B-1]
            A_even = bmat_sb[:, CH:3 * CH]       # [B0 | B-1]
            A_odd = bmat_sb[:, 0:2 * CH]         # [B+1 | B0]
            B_plus = bmat_sb[:, 0:CH]            # B+1 (from even source s -> chunk s-1)
            B_minus = bmat_sb[:, 2 * CH:3 * CH]  # B-1 (from odd source s -> chunk s+1)

            for img in range(NPC):
                # ---- stage A: load x, cast fp32->f16 during DMA ----
                # 16 pad cols so transpose stationaries can read 128 cols (FWL)
                x_sb = xp.tile([128, CT, HW + 16], f16, tag="x")
                for ci in range(CT):
                    nc.vector.memset(x_sb[:, ci, HW:], 0.0)
                xflats = [x_d[img, 128 * ci:128 * (ci + 1), :, :].rearrange("c h w -> c (h w)")
                          for ci in range(CT)]
                QTR = HW // 4
                for hh in range(4):      # quarters, both c-tiles interleaved,
                    for ci in range(CT):  # so the first transposes start early
                        nc.scalar.dma_start(
                            out=x_sb[:, ci, QTR * hh: QTR * (hh + 1)],
                            in_=xflats[ci][:, QTR * hh: QTR * (hh + 1)],
                        )

                # ---- stage B: transpose (x^T scaled by s1 via block-diag rhs) ----
                xt_sb = xtp.tile([CH, NCHUNK * C], f16, tag="xt")
                for q in range(NPAIR):
                    # stationary reads 128 cols (112 real + 16 overlap/pad) so
                    # FWL engages; psum rows 112:128 are written but never read
                    t_ps = tps.tile([128, 512], f32, tag="tps")
                    for k in range(2):          # chunk s = 2q+k
                        s = 2 * q + k
                        for ci in range(CT):
                            nc.tensor.matmul(
                                t_ps[:, 256 * k + 128 * ci: 256 * k + 128 * (ci + 1)],
                                lhsT=x_sb[:, ci, CH * s: CH * s + 128],
                                rhs=sdiag_sb[:, 128 * ci:128 * (ci + 1)],
                                start=True, stop=True,
                            )
                    epilogue(xt_sb[:, 512 * q: 512 * (q + 1)], t_ps[0:CH, :], None)

                def xt(s, ci):
                    return xt_sb[:, 256 * s + 128 * ci: 256 * s + 128 * (ci + 1)]

                # ---- stage C: depthwise conv, scatter form over sources ----
                # combined psum tile g covers chunks 4g..4g+3 as [128, bank 2, 256]
                y_sb = yp.tile([128, CT, HW], f16, tag="y")
                NG = NCHUNK // 4  # 7
                B_full = bmat_sb[:, 0:3 * CH]  # [B+1 | B0 | B-1], 336 wide
                for ci in range(CT):
                    for g in range(NG):
                        # one 2 KiB PSUM bank holds 4 chunks (448 of 512 f32);
                        # interior sources then land their full 336-wide
                        # [B+1|B0|B-1] window in a single matmul. Every MM
                        # overlaps its predecessor's range, which both orders
                        # them and makes the per-element accumulate correct.
                        cv = cvs.tile([128, 512], f32, tag="cv")
                        mms = []  # (src_chunk, rhs, lo, hi)
                        if g > 0:
                            mms.append((4 * g - 1, B_minus, 0, CH))
                        mms.append((4 * g, A_even, 0, 2 * CH))
                        mms.append((4 * g + 1, B_full, 0, 3 * CH))
                        mms.append((4 * g + 2, B_full, CH, 4 * CH))
                        mms.append((4 * g + 3, A_odd, 2 * CH, 4 * CH))
                        if g < NG - 1:
                            mms.append((4 * g + 4, B_plus, 3 * CH, 4 * CH))
                        for i, (s, rhs, lo, hi) in enumerate(mms):
                            nc.tensor.matmul(
                                cv[:, lo:hi], lhsT=xt(s, ci), rhs=rhs,
                                start=(i == 0), stop=(i == len(mms) - 1),
                                skip_group_check=True,
                            )
                        epilogue(
                            y_sb[:, ci, 448 * g: 448 * (g + 1)],
                            cv[:, 0:4 * CH],
                            t1_sb[:, ci:ci + 1],
                        )

                # ---- stage D: pointwise GEMM + BN2+ReLU (s2 folded into W) ----
                for oi in range(OT):
                    z_sb = zp.tile([128, HW], f16, tag="z")
                    zflat = z_d[img, 128 * oi:128 * (oi + 1), :, :].rearrange("o h w -> o (h w)")
                    for pb in range(NPB):
                        z_ps = zps.tile([128, GN], f32, tag="zps")
                        for ci in range(CT):
                            nc.tensor.matmul(
                                z_ps,
                                lhsT=w_sb[:, ci, 128 * oi:128 * (oi + 1)],
                                rhs=y_sb[:, ci, GN * pb: GN * (pb + 1)],
                                start=(ci == 0), stop=(ci == CT - 1),
                            )
                        epilogue(z_sb[:, GN * pb: GN * (pb + 1)], z_ps,
                                 t2_sb[:, oi:oi + 1])
                        # ---- stage E: two half-DMAs per otile (few descriptors,
                        # still overlaps the second half's compute) ----
                        if pb == 3:
                            nc.sync.dma_start(out=zflat[:, 0:4 * GN], in_=z_sb[:, 0:4 * GN])
                        elif pb == NPB - 1:
                            nc.sync.dma_start(out=zflat[:, 4 * GN:], in_=z_sb[:, 4 * GN:])

    nc.compile()
    return nc


def _build_bmats(k2d):
    """B[t][p,j] = k2d[1+dh, 1+dw] with dh = 2t + p//56 - j//56, dw = p%56 - j%56."""
    p = np.arange(CH)
    j = np.arange(CH)
    ph, pw = p // W, p % W
    jh, jw = j // W, j % W
    out = []
    for t in (1, 0, -1):  # concat order [B_{+1} | B_0 | B_{-1}]
        dh = 2 * t + ph[:, None] - jh[None, :]
        dw = pw[:, None] - jw[None, :]
        ok = (np.abs(dh) <= 1) & (np.abs(dw) <= 1)
        B = np.where(ok, k2d[np.clip(1 + dh, 0, 2), np.clip(1 + dw, 0, 2)], 0.0)
        out.append(B)
    return np.concatenate(out, axis=1).astype(np.float32)  # [112, 336]


def _get_runner(nc):
    """Build a cached jitted shard_map executable mirroring
    concourse.bass2jax.run_bass_via_pjrt (which re-traces on every call).

    Buffer protocol: all operands (inputs AND the dummy output-storage
    buffers) are pre-shardable with NamedSharding(mesh, P("core")) and
    are NOT donated.  Properly-sharded resident device buffers are not
    re-shipped through the axon tunnel on each execute, and without
    donation one zero output-storage set can be reused for every call
    (the NEFF writes the full output into the XLA result buffer, so the
    operand's contents are never read).  This removes ~25 MB/call of
    tunnel traffic vs. donated host-built zero buffers."""
    import jax
    import concourse.mybir as mybir
    from jax.sharding import Mesh, PartitionSpec, NamedSharding
    from jax.experimental.shard_map import shard_map
    from concourse.bass2jax import (
        _bass_exec_p, install_neuronx_cc_hook, partition_id_tensor)

    install_neuronx_cc_hook()

    partition_name = nc.partition_id_tensor.name if nc.partition_id_tensor else None

    in_names, out_names, out_avals = [], [], []
    for alloc in nc.m.functions[0].allocations:
        if not isinstance(alloc, mybir.MemoryLocationSet):
            continue
        name = alloc.memorylocations[0].name
        if alloc.kind == "ExternalInput":
            if name != partition_name:
                in_names.append(name)
        elif alloc.kind == "ExternalOutput":
            out_names.append(name)
            out_avals.append(jax.core.ShapedArray(
                tuple(alloc.tensor_shape), mybir.dt.np(alloc.dtype)))
    n_params = len(in_names)
    all_names = list(in_names) + list(out_names)
    if partition_name is not None:
        all_names.append(partition_name)

    def _body(*args):
        operands = list(args)
        if partition_name is not None:
            operands.append(partition_id_tensor())
        return tuple(_bass_exec_p.bind(
            *operands,
            out_avals=tuple(out_avals),
            in_names=tuple(all_names),
            out_names=tuple(out_names),
            lowering_input_output_aliases=(),
            sim_require_finite=True,
            sim_require_nnan=True,
            nc=nc,
        ))

    n_outs = len(out_avals)
    devices = jax.devices()[:N_CORES]
    mesh = Mesh(np.asarray(devices), ("core",))
    fn = jax.jit(
        shard_map(
            _body, mesh=mesh,
            in_specs=(PartitionSpec("core"),) * (n_params + n_outs),
            out_specs=(PartitionSpec("core"),) * len(out_names),
            check_rep=False,
        ),
        keep_unused=True,
    )
    out_shapes = [(N_CORES * a.shape[0], *a.shape[1:]) for a in out_avals]
    out_dtypes = [a.dtype for a in out_avals]
    sharding = NamedSharding(mesh, PartitionSpec("core"))
    return fn, in_names, out_names, out_shapes, out_dtypes, sharding


def _get_zero_outs():
    """One reusable, device-resident, core-sharded zero buffer set for
    output storage (never donated, never read — see _get_runner)."""
    import jax
    if "zero_outs" not in _cache:
        _, _, _, out_shapes, out_dtypes, sharding = _cache["runner"]
        _cache["zero_outs"] = [
            jax.device_put(np.zeros(s, d), sharding)
            for s, d in zip(out_shapes, out_dtypes)]
        jax.block_until_ready(_cache["zero_outs"])
    return _cache["zero_outs"]


def _prep_inputs(x, w_dw, b_dw, bn1_gamma, bn1_beta, bn1_mean, bn1_var,
                 w_pw, b_pw, bn2_gamma, bn2_beta, bn2_mean, bn2_var):
    bf = np.float16  # fp16 on the wire and on-chip: halves HBM traffic vs fp32

    x = np.asarray(x, np.float32)
    s1 = (bn1_gamma / np.sqrt(bn1_var + EPS)).astype(np.float32)
    t1 = (bn1_beta - bn1_mean * s1 + s1 * float(np.asarray(b_dw).reshape(-1)[0])).astype(np.float32)
    s2 = (bn2_gamma / np.sqrt(bn2_var + EPS)).astype(np.float32)
    t2 = (bn2_beta - bn2_mean * s2 + s2 * np.asarray(b_pw, np.float32)).astype(np.float32)

    # s1 applied during the transpose matmul (block-diag rhs);
    # s2 folded into the pointwise weights (inside relu arg, sign-free).
    sdiag = np.zeros((128, CT * 128), np.float32)
    for ci in range(CT):
        sdiag[:, 128 * ci:128 * (ci + 1)] = np.diag(s1[128 * ci:128 * (ci + 1)])
    wS = np.asarray(w_pw, np.float32) * s2[:, None]          # [CO, C]

    shared = {
        "bmat": _build_bmats(np.asarray(w_dw, np.float32)[0, 0]).astype(bf),
        "sdiag": sdiag.astype(bf),
        "wT": np.ascontiguousarray(wS.T).astype(bf),
        "t1": np.ascontiguousarray(t1.reshape(CT, 128).T),
        "t2": np.ascontiguousarray(t2.reshape(OT, 128).T),
    }
    return [{"x": np.ascontiguousarray(x[NPC * i: NPC * (i + 1)]).astype(np.float16),
             **shared} for i in range(N_CORES)]


def kernel(x, w_dw, b_dw, bn1_gamma, bn1_beta, bn1_mean, bn1_var,
           w_pw, b_pw, bn2_gamma, bn2_beta, bn2_mean, bn2_var):
    if "nc" not in _cache:
        _cache["nc"] = _build_program()
    nc = _cache["nc"]

    in_maps = _prep_inputs(x, w_dw, b_dw, bn1_gamma, bn1_beta, bn1_mean, bn1_var,
                           w_pw, b_pw, bn2_gamma, bn2_beta, bn2_mean, bn2_var)
    try:
        # cached-jit PJRT path only under axon (native NRT boxes take the
        # run_bass_kernel_spmd path below, which drives /dev/neuron* directly)
        import jax
        from concourse._compat import axon_active
        if not axon_active():
            raise RuntimeError("native NRT environment")
        if "runner" not in _cache:
            _cache["runner"] = _get_runner(nc)
        fn, in_names, out_names, out_shapes, out_dtypes, sharding = _cache["runner"]
        concat_in = [np.concatenate([m[name] for m in in_maps], axis=0)
                     for name in in_names]
        dev_in = [jax.device_put(a, sharding) for a in concat_in]
        outs = fn(*dev_in, *_get_zero_outs())
        z = np.asarray(outs[out_names.index("z")])
        return z.astype(np.float32)
    except Exception:
        # fallback + retry: rare transient NRT_EXEC_UNIT_UNRECOVERABLE errors
        # have been observed on first device contact
        import time
        from concourse import bass_utils
        last_exc = None
        for attempt in range(3):
            try:
                res = bass_utils.run_bass_kernel_spmd(
                    nc, in_maps, core_ids=list(range(N_CORES)))
                z = np.concatenate([res.results[i]["z"] for i in range(N_CORES)], axis=0)
                return np.asarray(z, np.float32)
            except Exception as e:
                last_exc = e
                time.sleep(2.0 * (attempt + 1))
        raise last_exc


def _kernel_spmd_reference_path(x, w_dw, b_dw, bn1_gamma, bn1_beta, bn1_mean, bn1_var,
                                w_pw, b_pw, bn2_gamma, bn2_beta, bn2_mean, bn2_var):
    """Original path through bass_utils.run_bass_kernel_spmd (kept for checking)."""
    from concourse import bass_utils

    if "nc" not in _cache:
        _cache["nc"] = _build_program()
    nc = _cache["nc"]

    in_maps = _prep_inputs(x, w_dw, b_dw, bn1_gamma, bn1_beta, bn1_mean, bn1_var,
                           w_pw, b_pw, bn2_gamma, bn2_beta, bn2_mean, bn2_var)
    res = bass_utils.run_bass_kernel_spmd(nc, in_maps, core_ids=list(range(N_CORES)))
    z = np.concatenate([res.results[i]["z"] for i in range(N_CORES)], axis=0)
    return np.asarray(z, np.float32)



# revision 5
# speedup vs baseline: 171.8355x; 1.2002x over previous
"""Trainium2 Bass kernel for DepthWiseSeparableConv (shared-3x3 dw conv + BN+ReLU + 1x1 conv + BN+ReLU).

Strategy (8 NeuronCores, data-parallel over batch N=32 -> 4 images/core):
  - Host pre-work: x is BN1-scaled (fp32, exact) and TRANSPOSED to
    [img, pix, c] fp16 before upload, so the device never runs the PE
    transpose stage; z returns fp16 and is upcast on host.
  - Depthwise 3x3 conv as banded matmuls: one source chunk (112 px = 2
    rows) contributes a CONTIGUOUS 224-px window [prev-row 56 | own 112 |
    next-row 56] of targets, so each interior source needs a single
    224-wide matmul (2 moving cols/output px; the old [B+1|B0|B-1] form
    streamed 3).  PSUM bank = 4 chunks (448 px); boundary sources split.
  - BN1 shift + ReLU epilogue: one op (bias-add + max0), weighted
    alternation between VectorE and ScalarE (vector is faster; scalar
    also absorbs semaphores), cast to fp16 -> y [c, pix].
  - 1x1 conv = GEMM over 2 c-tiles; BN2's scale folded into the weights
    (inside the relu argument, so no sign assumption), shift+ReLU as 1 op.
  - input DMA rides the GpSimd HWDGE ring (keeps ScalarE free for
    epilogues), outputs + singles the SP ring.

Self-contained: hardcodes all shapes; no file reads.
"""

import numpy as np

N, C, CO, H, W = 32, 256, 512, 56, 56
EPS = 1e-5
N_CORES = 8
NPC = N // N_CORES      # images per core
HW = H * W              # 3136
CH = 112                # pixel chunk = 2 rows of 56
NCHUNK = HW // CH       # 28
CT = C // 128           # 2 c-tiles
OT = CO // 128          # 4 o-tiles
GN = 448                # gemm pixel-block == conv psum group (4 chunks)
NPB = HW // GN          # 7
NG = NCHUNK // 4        # 7 conv groups

_cache = {}


def _build_program():
    import concourse.mybir as mybir
    import concourse.tile as tile
    from concourse import bacc

    f32 = mybir.dt.float32
    f16 = mybir.dt.float16

    nc = bacc.Bacc("TRN2", target_bir_lowering=False, debug=False)

    xT_d = nc.dram_tensor("xT", [NPC, HW, C], f16, kind="ExternalInput").ap()
    bmat_d = nc.dram_tensor("bmat", [CH, 2 * CH], f16, kind="ExternalInput").ap()
    wT_d = nc.dram_tensor("wT", [C, CO], f16, kind="ExternalInput").ap()  # s2-folded
    t1_d = nc.dram_tensor("t1", [128, CT], f32, kind="ExternalInput").ap()
    t2_d = nc.dram_tensor("t2", [128, OT], f32, kind="ExternalInput").ap()
    z_d = nc.dram_tensor("z", [NPC, CO, H, W], f16, kind="ExternalOutput").ap()

    relu = mybir.ActivationFunctionType.Relu
    add = mybir.AluOpType.add
    amax = mybir.AluOpType.max

    with tile.TileContext(nc) as tc:
        with (
            tc.tile_pool(name="singles", bufs=1) as singles,
            tc.tile_pool(name="xtp", bufs=3) as xtp,
            tc.tile_pool(name="yp", bufs=3) as yp,
            tc.tile_pool(name="zp", bufs=6) as zp,
            tc.tile_pool(name="cvs", bufs=3, space="PSUM") as cvs,
            tc.tile_pool(name="zps", bufs=3, space="PSUM") as zps,
        ):
            bmat_sb = singles.tile([CH, 2 * CH], f16)
            nc.sync.dma_start(out=bmat_sb, in_=bmat_d)
            w_sb = singles.tile([128, CT, CO], f16)
            for ci in range(CT):
                nc.sync.dma_start(out=w_sb[:, ci, :], in_=wT_d[128 * ci:128 * (ci + 1), :])
            t1_sb = singles.tile([128, CT], f32)
            nc.sync.dma_start(out=t1_sb, in_=t1_d)
            t2_sb = singles.tile([128, OT], f32)
            nc.sync.dma_start(out=t2_sb, in_=t2_d)

            # epilogue helper: out = relu(in + bias[p]); 3:2 vector:scalar
            epi_ctr = [0]

            def epilogue(out_ap, in_ap, bias_ap):
                use_vec = (epi_ctr[0] % 5) < 3
                epi_ctr[0] += 1
                if use_vec:
                    nc.vector.tensor_scalar(out=out_ap, in0=in_ap,
                                            scalar1=bias_ap, scalar2=0.0,
                                            op0=add, op1=amax)
                else:
                    nc.scalar.activation(out=out_ap, in_=in_ap, func=relu,
                                         bias=bias_ap, scale=1.0)

            for img in range(NPC):
                # ---- stage A: load x^T (pre-scaled by s1 on host) ----
                xt_sb = xtp.tile([CH, NCHUNK, C], f16, tag="xt")
                xflat = xT_d[img].rearrange("(s p) c -> p s c", p=CH)
                for q in range(NG):
                    nc.gpsimd.dma_start(out=xt_sb[:, 4 * q:4 * (q + 1), :],
                                        in_=xflat[:, 4 * q:4 * (q + 1), :])

                def xt(s, ci):
                    return xt_sb[:, s, 128 * ci:128 * (ci + 1)]

                # ---- stage B: depthwise conv, contiguous-window scatter ----
                # source chunk s covers flat targets [112s-56, 112s+168):
                # bmat[:, q] is the weight of source px j onto target
                # 112s-56+q.  PSUM group g = 4 chunks = [448g, 448g+448).
                y_sb = yp.tile([128, CT, HW], f16, tag="y")
                for ci in range(CT):
                    for g in range(NG):
                        cv = cvs.tile([128, 512], f32, tag="cv")
                        mms = []  # (src chunk, q-lo, q-hi, out-lo)
                        if g > 0:
                            mms.append((4 * g - 1, 168, 224, 0))
                        mms.append((4 * g, 56, 224, 0))
                        mms.append((4 * g + 1, 0, 224, 56))
                        mms.append((4 * g + 2, 0, 224, 168))
                        mms.append((4 * g + 3, 0, 168, 280))
                        if g < NG - 1:
                            mms.append((4 * g + 4, 0, 56, 392))
                        for i, (s, qlo, qhi, olo) in enumerate(mms):
                            nc.tensor.matmul(
                                cv[:, olo:olo + (qhi - qlo)],
                                lhsT=xt(s, ci),
                                rhs=bmat_sb[:, qlo:qhi],
                                start=(i == 0), stop=(i == len(mms) - 1),
                                skip_group_check=True,
                            )
                        epilogue(
                            y_sb[:, ci, GN * g: GN * (g + 1)],
                            cv[:, 0:GN],
                            t1_sb[:, ci:ci + 1],
                        )

                # ---- stage C: pointwise GEMM + BN2+ReLU (s2 folded into W) ----
                last_img = img == NPC - 1
                for oi in range(OT):
                    z_sb = zp.tile([128, HW], f16, tag="z")
                    zflat = z_d[img, 128 * oi:128 * (oi + 1), :, :].rearrange("o h w -> o (h w)")
                    for pb in range(NPB):
                        z_ps = zps.tile([128, GN], f32, tag="zps")
                        for ci in range(CT):
                            nc.tensor.matmul(
                                z_ps,
                                lhsT=w_sb[:, ci, 128 * oi:128 * (oi + 1)],
                                rhs=y_sb[:, ci, GN * pb: GN * (pb + 1)],
                                start=(ci == 0), stop=(ci == CT - 1),
                            )
                        epilogue(z_sb[:, GN * pb: GN * (pb + 1)], z_ps,
                                 t2_sb[:, oi:oi + 1])
                        # ---- stage D: output DMA; finer splits on the last
                        # otile so the drain tail stays short ----
                        if last_img and oi == OT - 1:
                            if pb in (1, 3, 5):
                                nc.sync.dma_start(
                                    out=zflat[:, GN * (pb - 1):GN * (pb + 1)],
                                    in_=z_sb[:, GN * (pb - 1):GN * (pb + 1)])
                            elif pb == NPB - 1:
                                nc.sync.dma_start(
                                    out=zflat[:, GN * pb:],
                                    in_=z_sb[:, GN * pb:])
                        elif pb == 3:
                            nc.sync.dma_start(out=zflat[:, 0:4 * GN], in_=z_sb[:, 0:4 * GN])
                        elif pb == NPB - 1:
                            nc.sync.dma_start(out=zflat[:, 4 * GN:], in_=z_sb[:, 4 * GN:])

    nc.compile()
    return nc


def _build_bmat2(k2d):
    """bmat[j, q] = weight of source px j (within a 112-px chunk) onto the
    flat target px (112s - 56 + q) for any source chunk s:
    dh = q//56 - 1 - j//56, dw = q%56 - j%56, weight k2d[1+dh, 1+dw]."""
    j = np.arange(CH)
    q = np.arange(2 * CH)
    dh = q[None, :] // 56 - 1 - (j[:, None] // 56)
    dw = (q[None, :] % 56) - (j[:, None] % 56)
    ok = (np.abs(dh) <= 1) & (np.abs(dw) <= 1)
    # jax's conv_general_dilated is cross-correlation: the weight of source
    # px j on target px q is k2d[1 - dh, 1 - dw] (source minus target).
    B = np.where(ok, k2d[np.clip(1 - dh, 0, 2), np.clip(1 - dw, 0, 2)], 0.0)
    return B.astype(np.float32)  # [112, 224]


def _get_runner(nc):
    """Build a cached jitted shard_map executable mirroring
    concourse.bass2jax.run_bass_via_pjrt (which re-traces on every call).

    Buffer protocol: all operands (inputs AND the dummy output-storage
    buffers) are pre-shardable with NamedSharding(mesh, P("core")) and
    are NOT donated.  Properly-sharded resident device buffers are not
    re-shipped through the axon tunnel on each execute, and without
    donation one zero output-storage set can be reused for every call
    (the NEFF writes the full output into the XLA result buffer, so the
    operand's contents are never read)."""
    import jax
    import concourse.mybir as mybir
    from jax.sharding import Mesh, PartitionSpec, NamedSharding
    from jax.experimental.shard_map import shard_map
    from concourse.bass2jax import (
        _bass_exec_p, install_neuronx_cc_hook, partition_id_tensor)

    install_neuronx_cc_hook()

    partition_name = nc.partition_id_tensor.name if nc.partition_id_tensor else None

    in_names, out_names, out_avals = [], [], []
    for alloc in nc.m.functions[0].allocations:
        if not isinstance(alloc, mybir.MemoryLocationSet):
            continue
        name = alloc.memorylocations[0].name
        if alloc.kind == "ExternalInput":
            if name != partition_name:
                in_names.append(name)
        elif alloc.kind == "ExternalOutput":
            out_names.append(name)
            out_avals.append(jax.core.ShapedArray(
                tuple(alloc.tensor_shape), mybir.dt.np(alloc.dtype)))
    n_params = len(in_names)
    all_names = list(in_names) + list(out_names)
    if partition_name is not None:
        all_names.append(partition_name)

    def _body(*args):
        operands = list(args)
        if partition_name is not None:
            operands.append(partition_id_tensor())
        return tuple(_bass_exec_p.bind(
            *operands,
            out_avals=tuple(out_avals),
            in_names=tuple(all_names),
            out_names=tuple(out_names),
            lowering_input_output_aliases=(),
            sim_require_finite=True,
            sim_require_nnan=True,
            nc=nc,
        ))

    n_outs = len(out_avals)
    devices = jax.devices()[:N_CORES]
    mesh = Mesh(np.asarray(devices), ("core",))
    fn = jax.jit(
        shard_map(
            _body, mesh=mesh,
            in_specs=(PartitionSpec("core"),) * (n_params + n_outs),
            out_specs=(PartitionSpec("core"),) * len(out_names),
            check_rep=False,
        ),
        keep_unused=True,
    )
    out_shapes = [(N_CORES * a.shape[0], *a.shape[1:]) for a in out_avals]
    out_dtypes = [a.dtype for a in out_avals]
    sharding = NamedSharding(mesh, PartitionSpec("core"))
    return fn, in_names, out_names, out_shapes, out_dtypes, sharding


def _get_zero_outs():
    """One reusable, device-resident, core-sharded zero buffer set for
    output storage (never donated, never read — see _get_runner)."""
    import jax
    if "zero_outs" not in _cache:
        _, _, _, out_shapes, out_dtypes, sharding = _cache["runner"]
        _cache["zero_outs"] = [
            jax.device_put(np.zeros(s, d), sharding)
            for s, d in zip(out_shapes, out_dtypes)]
        jax.block_until_ready(_cache["zero_outs"])
    return _cache["zero_outs"]


def _prep_inputs(x, w_dw, b_dw, bn1_gamma, bn1_beta, bn1_mean, bn1_var,
                 w_pw, b_pw, bn2_gamma, bn2_beta, bn2_mean, bn2_var):
    bf = np.float16  # fp16 on the wire and on-chip: halves HBM traffic vs fp32

    x = np.asarray(x, np.float32)
    s1 = (bn1_gamma / np.sqrt(bn1_var + EPS)).astype(np.float32)
    t1 = (bn1_beta - bn1_mean * s1 + s1 * float(np.asarray(b_dw).reshape(-1)[0])).astype(np.float32)
    s2 = (bn2_gamma / np.sqrt(bn2_var + EPS)).astype(np.float32)
    t2 = (bn2_beta - bn2_mean * s2 + s2 * np.asarray(b_pw, np.float32)).astype(np.float32)

    # s1 applied to x on the host (exact, fp32) so the device consumes a
    # pre-scaled, pre-transposed x^T and skips the PE transpose stage;
    # s2 folded into the pointwise weights (inside relu arg, sign-free).
    xs = x.reshape(N, C, HW) * s1[None, :, None]
    xT = np.ascontiguousarray(xs.transpose(0, 2, 1)).astype(bf)  # [N, HW, C]
    wS = np.asarray(w_pw, np.float32) * s2[:, None]          # [CO, C]

    shared = {
        "bmat": _build_bmat2(np.asarray(w_dw, np.float32)[0, 0]).astype(bf),
        "wT": np.ascontiguousarray(wS.T).astype(bf),
        "t1": np.ascontiguousarray(t1.reshape(CT, 128).T),
        "t2": np.ascontiguousarray(t2.reshape(OT, 128).T),
    }
    return [{"xT": xT[NPC * i: NPC * (i + 1)], **shared} for i in range(N_CORES)]


def kernel(x, w_dw, b_dw, bn1_gamma, bn1_beta, bn1_mean, bn1_var,
           w_pw, b_pw, bn2_gamma, bn2_beta, bn2_mean, bn2_var):
    if "nc" not in _cache:
        _cache["nc"] = _build_program()
    nc = _cache["nc"]

    in_maps = _prep_inputs(x, w_dw, b_dw, bn1_gamma, bn1_beta, bn1_mean, bn1_var,
                           w_pw, b_pw, bn2_gamma, bn2_beta, bn2_mean, bn2_var)
    try:
        # cached-jit PJRT path only under axon (native NRT boxes take the
        # run_bass_kernel_spmd path below, which drives /dev/neuron* directly)
        import jax
        from concourse._compat import axon_active
        if not axon_active():
            raise RuntimeError("native NRT environment")
        if "runner" not in _cache:
            _cache["runner"] = _get_runner(nc)
        fn, in_names, out_names, out_shapes, out_dtypes, sharding = _cache["runner"]
        concat_in = [np.concatenate([m[name] for m in in_maps], axis=0)
                     for name in in_names]
        dev_in = [jax.device_put(a, sharding) for a in concat_in]
        outs = fn(*dev_in, *_get_zero_outs())
        z = np.asarray(outs[out_names.index("z")])
        return z.astype(np.float32)
    except Exception:
        # fallback + retry: rare transient NRT_EXEC_UNIT_UNRECOVERABLE errors
        # have been observed on first device contact
        import time
        from concourse import bass_utils
        last_exc = None
        for attempt in range(3):
            try:
                res = bass_utils.run_bass_kernel_spmd(
                    nc, in_maps, core_ids=list(range(N_CORES)))
                z = np.concatenate([res.results[i]["z"] for i in range(N_CORES)], axis=0)
                return np.asarray(z, np.float32)
            except Exception as e:
                last_exc = e
                time.sleep(2.0 * (attempt + 1))
        raise last_exc
